# revision 14
# baseline (speedup 1.0000x reference)
"""Trainium2 Bass kernel for a post-LN MultiHeadAttention block.

Problem: x:(4,2048,1024), 16 heads x 64 dims, returns (out, attn) where
  out  = LayerNorm(ctx @ w_o + x) : (4, 2048, 1024)
  attn = softmax(q k^T / 8)       : (4, 16, 2048, 2048)

Sharding: 8 cores = 4 batches x 2 query-halves.  Each core computes
K/V for its whole batch (duplicated across the 2 half-cores) and
queries/attention/output for its 1024 query rows.  No collectives.

On-chip dataflow (all fp32, matmuls in float32r = full-rate fp32):
  - x is PE-transposed once into feature-major XT.
  - QT/KT feature-major via matmul(lhsT=W, rhs=XT); V seq-major via
    matmul(lhsT=XT, rhs=w_v) with a ones-column per head appended.
  - Scores are computed transposed, S^T[kk,q] (softmax along
    partitions); exp on ACT; the softmax denominator falls out of the
    PV matmul through the ones column (row 64 of the [65,q] psum).
  - attn is written to HBM transposed (h, kk, q); the host transposes
    while gathering.
  - Output proj seq-major via matmul(lhsT=ctxT, rhs=w_o), fused
    residual + LayerNorm (bn_stats/bn_aggr) on-chip.
"""

import sys

sys.path.insert(0, "/opt/trn_rl_repo")

from contextlib import ExitStack

import numpy as np

import concourse.bacc as bacc
import concourse.bass as bass
import concourse.tile as tile
from concourse import mybir
from concourse.alu_op_type import AluOpType
from concourse.bass_utils import run_bass_kernel_spmd
from concourse.masks import make_identity

FP32 = mybir.dt.float32
FP32R = mybir.dt.float32r
AF = mybir.ActivationFunctionType

B, S, D = 4, 2048, 1024
H, DK = 16, 64
NQ = S // 2          # query rows per core
P = 128
NF = D // P          # 8 feature/dmodel tiles
NS = S // P          # 16 seq tiles (kk)
NSQ = NQ // P        # 8 query seq tiles
QC = 512             # query chunk (matmul moving dim)
NQC = NQ // QC       # 2 query chunks
LN_EPS = 1e-6

_CACHE = {}


def r(ap):
    return ap if ap.dtype == FP32R else ap.bitcast(FP32R)


def _build():
    nc = bacc.Bacc("TRN2", target_bir_lowering=False, debug=False, num_devices=8)

    x_kv = nc.dram_tensor("x_kv", [S, D], FP32, kind="ExternalInput").ap()
    x_q = nc.dram_tensor("x_q", [NQ, D], FP32, kind="ExternalInput").ap()
    w_q = nc.dram_tensor("w_q", [D, D], FP32, kind="ExternalInput").ap()
    w_k = nc.dram_tensor("w_k", [D, D], FP32, kind="ExternalInput").ap()
    w_v = nc.dram_tensor("w_v", [D, D], FP32, kind="ExternalInput").ap()
    w_o = nc.dram_tensor("w_o", [D, D], FP32, kind="ExternalInput").ap()
    gam = nc.dram_tensor("ln_gamma", [D], FP32, kind="ExternalInput").ap()
    bet = nc.dram_tensor("ln_beta", [D], FP32, kind="ExternalInput").ap()

    # attn, stored transposed: [h, kk-tile, kk-within, q]
    attn_t = nc.dram_tensor("attn_t", [H, NS, P, NQ], FP32, kind="ExternalOutput").ap()
    out_q = nc.dram_tensor("out_q", [NQ, D], FP32, kind="ExternalOutput").ap()

    # DRAM scratch (per-core local)
    v_scr = nc.dram_tensor("v_scr", [H, 4, P, NS // 4, DK + 1], FP32R).ap()
    qt_scr = nc.dram_tensor("qt_scr", [NF, P, NQ], FP32R).ap()
    ctx_scr = nc.dram_tensor("ctx_scr", [NF, P, NQ], FP32R).ap()

    with ExitStack() as ctx:
        tc = ctx.enter_context(tile.TileContext(nc))

        const = ctx.enter_context(tc.tile_pool(name="const", bufs=1))
        ident = const.tile([P, P], FP32)
        make_identity(nc, ident)

        # KT stays in SBUF until the end of attention (outermost pool).
        kt_pool = ctx.enter_context(tc.tile_pool(name="kt", bufs=1))
        KT = [kt_pool.tile([P, S], FP32R, tag=f"kt{f}", name=f"KT{f}") for f in range(NF)]

        # ---- Phase A0q/A1: XTq via PE transpose, QT -> DRAM, free XTq ----
        with tc.tile_pool(name="xq_feat", bufs=1) as xtq_pool, \
             tc.tile_pool(name="xin", bufs=3) as xin_pool, \
             tc.tile_pool(name="tp_ps", bufs=4, space="PSUM") as tp_ps, \
             tc.tile_pool(name="wts", bufs=1) as w_pool, \
             tc.tile_pool(name="mm_ps", bufs=4, space="PSUM") as mm_ps, \
             tc.tile_pool(name="stage", bufs=3) as stage_pool:

            XTq = [xtq_pool.tile([P, NQ], FP32R, tag=f"xtq{f}", name=f"XTq{f}")
                   for f in range(NF)]
            for i in range(NSQ):
                xin = xin_pool.tile([P, D], FP32)
                nc.sync.dma_start(out=xin, in_=x_q[i * P:(i + 1) * P, :])
                for f in range(NF):
                    ps = tp_ps.tile([P, P], FP32)
                    nc.tensor.transpose(ps, xin[:, f * P:(f + 1) * P], ident)
                    nc.vector.tensor_copy(out=XTq[f][:, i * P:(i + 1) * P], in_=ps)

            wq_sb = w_pool.tile([P, NF, D], FP32R, tag="w")
            nc.sync.dma_start(out=wq_sb, in_=r(w_q.rearrange("(k p) f -> p k f", p=P)))
            for f in range(NF):
                qt_t = stage_pool.tile([P, NQ], FP32R, tag="qt_stage")
                for n in range(NQC):
                    ps = mm_ps.tile([P, QC], FP32)
                    for k in range(NF):
                        nc.tensor.matmul(
                            ps,
                            lhsT=r(wq_sb[:, k, f * P:(f + 1) * P]),
                            rhs=r(XTq[k][:, n * QC:(n + 1) * QC]),
                            start=(k == 0),
                            stop=(k == NF - 1),
                        )
                    nc.vector.tensor_copy(out=qt_t[:, n * QC:(n + 1) * QC], in_=ps)
                nc.sync.dma_start(out=qt_scr[f], in_=qt_t)

        # ---- Phases A0kv/A2/A3: XT, then KT (resident) and V -> DRAM ----
        with tc.tile_pool(name="xt", bufs=1) as xt_pool:
            XT = [xt_pool.tile([P, S], FP32R, tag=f"xt{f}", name=f"XT{f}")
                  for f in range(NF)]
            with tc.tile_pool(name="xin2", bufs=2) as xin_pool, \
                 tc.tile_pool(name="tp_ps2", bufs=4, space="PSUM") as tp_ps, \
                 tc.tile_pool(name="wts2", bufs=1) as w_pool, \
                 tc.tile_pool(name="mm_ps2", bufs=4, space="PSUM") as mm_ps, \
                 tc.tile_pool(name="stage2", bufs=3) as stage_pool:

                for i in range(NS):
                    xin = xin_pool.tile([P, D], FP32)
                    nc.sync.dma_start(out=xin, in_=x_kv[i * P:(i + 1) * P, :])
                    for f in range(NF):
                        ps = tp_ps.tile([P, P], FP32)
                        nc.tensor.transpose(ps, xin[:, f * P:(f + 1) * P], ident)
                        nc.vector.tensor_copy(out=XT[f][:, i * P:(i + 1) * P], in_=ps)

                wk_sb = w_pool.tile([P, NF, D], FP32R, tag="w")
                nc.sync.dma_start(out=wk_sb, in_=r(w_k.rearrange("(k p) f -> p k f", p=P)))
                for f in range(NF):
                    for n in range(S // QC):
                        ps = mm_ps.tile([P, QC], FP32)
                        for k in range(NF):
                            nc.tensor.matmul(
                                ps,
                                lhsT=r(wk_sb[:, k, f * P:(f + 1) * P]),
                                rhs=r(XT[k][:, n * QC:(n + 1) * QC]),
                                start=(k == 0),
                                stop=(k == NF - 1),
                            )
                        nc.vector.tensor_copy(out=KT[f][:, n * QC:(n + 1) * QC], in_=ps)

                wv_sb = w_pool.tile([P, NF, D], FP32R, tag="w")
                nc.sync.dma_start(out=wv_sb, in_=r(w_v.rearrange("(k p) f -> p k f", p=P)))
                NSH = NS // 4
                for half in range(4):
                    v_t = stage_pool.tile([P, NSH, H, DK + 1], FP32R, tag="v_stage", bufs=1)
                    for mm_ in range(NSH):
                        m = half * NSH + mm_
                        for n in range(2):
                            ps = mm_ps.tile([P, QC], FP32)
                            for k in range(NF):
                                nc.tensor.matmul(
                                    ps,
                                    lhsT=r(XT[k][:, m * P:(m + 1) * P]),
                                    rhs=r(wv_sb[:, k, n * QC:(n + 1) * QC]),
                                    start=(k == 0),
                                    stop=(k == NF - 1),
                                )
                            nc.vector.tensor_copy(
                                out=v_t[:, mm_, n * 8:(n + 1) * 8, 0:DK],
                                in_=ps.rearrange("p (h d) -> p h d", d=DK),
                            )
                    nc.vector.memset(v_t[:, :, :, DK:DK + 1].bitcast(FP32), 1.0)
                    for h in range(H):
                        nc.sync.dma_start(
                            out=v_scr[h, half], in_=v_t[:, :, h, :]
                        )

        # ---- Attention ----
        with tc.tile_pool(name="qt_pair", bufs=2) as qt_pool, \
             tc.tile_pool(name="vh", bufs=2) as vh_pool, \
             tc.tile_pool(name="slab", bufs=1) as slab_pool, \
             tc.tile_pool(name="norm", bufs=4) as norm_pool, \
             tc.tile_pool(name="recb", bufs=1) as recb_pool, \
             tc.tile_pool(name="ctxp", bufs=2) as ctxp_pool, \
             tc.tile_pool(name="st_ps", bufs=2, space="PSUM") as st_ps_pool, \
             tc.tile_pool(name="ctx_ps", bufs=1, space="PSUM") as ctx_ps_pool:

            for hp in range(H // 2):
                qt_p = qt_pool.tile([P, NQ], FP32R)
                nc.sync.dma_start(out=qt_p, in_=qt_scr[hp])
                ctx_pair = ctxp_pool.tile([P, NQ], FP32R)
                v_hs = []
                for hh in range(2):
                    h = 2 * hp + hh
                    v_h = vh_pool.tile([P, 4, NS // 4, DK + 1], FP32R,
                                       tag=f"vh{hh}", name=f"v_h{h}")
                    nc.sync.dma_start(
                        out=v_h, in_=v_scr[h].rearrange("x p m c -> p x m c")
                    )
                    v_hs.append(v_h.rearrange("p x m c -> p (x m) c"))
                for qc in range(NQC):
                    slabs = []
                    ctx_pss = []
                    for hh in range(2):
                        slabs.append(slab_pool.tile(
                            [P, NS, QC], FP32R, tag=f"slab{hh}", name=f"slab{hh}"))
                        ctx_pss.append(ctx_ps_pool.tile(
                            [DK + 1, QC], FP32, tag=f"ctx{hh}", name=f"ctx_ps{hh}"))
                    for m in range(NS):
                        for hh in range(2):
                            r0 = DK * hh
                            st_ps = st_ps_pool.tile([P, QC], FP32, tag=f"st{hh}")
                            nc.tensor.matmul(
                                st_ps,
                                lhsT=r(KT[hp][r0:r0 + DK, m * P:(m + 1) * P]),
                                rhs=r(qt_p[r0:r0 + DK, qc * QC:(qc + 1) * QC]),
                                start=True,
                                stop=True,
                            )
                            nc.scalar.activation(
                                out=slabs[hh][:, m, :], in_=st_ps,
                                func=AF.Exp, scale=0.125,
                            )
                            nc.tensor.matmul(
                                ctx_pss[hh],
                                lhsT=r(v_hs[hh][:, m, :]),
                                rhs=r(slabs[hh][:, m, :]),
                                start=(m == 0),
                                stop=(m == NS - 1),
                            )
                    for hh in range(2):
                        h = 2 * hp + hh
                        r0 = DK * hh
                        srow = recb_pool.tile([1, QC], FP32, tag=f"sr{hh}")
                        nc.scalar.copy(out=srow, in_=ctx_pss[hh][DK:DK + 1, :])
                        sumB = recb_pool.tile([P, QC], FP32, tag=f"sb{hh}")
                        nc.gpsimd.partition_broadcast(sumB, srow)
                        recB = recb_pool.tile([P, QC], FP32, tag=f"rb{hh}")
                        nc.vector.reciprocal(out=recB, in_=sumB)
                        nc.vector.tensor_tensor(
                            out=ctx_pair[r0:r0 + DK, qc * QC:(qc + 1) * QC],
                            in0=ctx_pss[hh][0:DK, :],
                            in1=recB[0:DK, :],
                            op=AluOpType.mult,
                        )
                        for m in range(NS):
                            nt = norm_pool.tile([P, QC], FP32, tag=f"nt{hh}")
                            nc.vector.tensor_tensor(
                                out=nt,
                                in0=slabs[hh][:, m, :],
                                in1=recB,
                                op=AluOpType.mult,
                            )
                            nc.sync.dma_start(
                                out=attn_t[h, m, :, qc * QC:(qc + 1) * QC],
                                in_=nt,
                            )
                nc.sync.dma_start(out=ctx_scr[hp], in_=ctx_pair)

        # ---- Output projection + residual + LayerNorm ----
        with tc.tile_pool(name="wts3", bufs=1) as w_pool, \
             tc.tile_pool(name="mm_ps3", bufs=4, space="PSUM") as mm_ps, \
             tc.tile_pool(name="ctxo", bufs=3) as ctxo_pool, \
             tc.tile_pool(name="y", bufs=3) as y_pool, \
             tc.tile_pool(name="xr", bufs=3) as xr_pool, \
             tc.tile_pool(name="lnst", bufs=4) as ln_pool:

            gamB = const.tile([P, D], FP32)
            nc.gpsimd.dma_start(
                out=gamB,
                in_=bass.AP(tensor=gam.tensor, offset=gam.offset, ap=[[0, P], [1, D]]),
            )
            betB = const.tile([P, D], FP32)
            nc.gpsimd.dma_start(
                out=betB,
                in_=bass.AP(tensor=bet.tensor, offset=bet.offset, ap=[[0, P], [1, D]]),
            )
            eps_t = const.tile([P, 1], FP32)
            nc.vector.memset(eps_t, LN_EPS)

            wo_sb = w_pool.tile([P, NF, D], FP32R, tag="w")
            nc.sync.dma_start(out=wo_sb, in_=r(w_o.rearrange("(k p) f -> p k f", p=P)))

            for qt in range(NSQ):
                ctxo = ctxo_pool.tile([P, NF, P], FP32R)
                nc.sync.dma_start(
                    out=ctxo,
                    in_=ctx_scr[:, :, qt * P:(qt + 1) * P].rearrange("k p q -> p k q"),
                )
                y = y_pool.tile([P, D], FP32)
                xr = xr_pool.tile([P, D], FP32)
                nc.sync.dma_start(out=xr, in_=x_q[qt * P:(qt + 1) * P, :])
                for n in range(2):
                    ps = mm_ps.tile([P, QC], FP32)
                    for k in range(NF):
                        nc.tensor.matmul(
                            ps,
                            lhsT=r(ctxo[:, k, :]),
                            rhs=r(wo_sb[:, k, n * QC:(n + 1) * QC]),
                            start=(k == 0),
                            stop=(k == NF - 1),
                        )
                    nc.vector.tensor_tensor(
                        out=y[:, n * QC:(n + 1) * QC],
                        in0=ps,
                        in1=xr[:, n * QC:(n + 1) * QC],
                        op=AluOpType.add,
                    )
                stats = ln_pool.tile([P, 2, nc.vector.BN_STATS_DIM], FP32, tag="stats")
                for g in range(2):
                    nc.vector.bn_stats(out=stats[:, g, :], in_=y[:, g * 512:(g + 1) * 512])
                mv = ln_pool.tile([P, nc.vector.BN_AGGR_DIM], FP32, tag="mv")
                nc.vector.bn_aggr(out=mv, in_=stats)
                sd = ln_pool.tile([P, 1], FP32, tag="sd")
                nc.scalar.activation(
                    out=sd, in_=mv[:, 1:2], func=AF.Sqrt, bias=eps_t, scale=1.0
                )
                rstd = ln_pool.tile([P, 1], FP32, tag="rstd")
                nc.vector.reciprocal(out=rstd, in_=sd)
                t = y_pool.tile([P, D], FP32, tag="t")
                nc.vector.tensor_scalar(
                    out=t,
                    in0=y,
                    scalar1=mv[:, 0:1],
                    scalar2=rstd,
                    op0=AluOpType.subtract,
                    op1=AluOpType.mult,
                )
                nc.vector.tensor_tensor(out=t, in0=t, in1=gamB, op=AluOpType.mult)
                nc.vector.tensor_tensor(out=t, in0=t, in1=betB, op=AluOpType.add)
                nc.sync.dma_start(out=out_q[qt * P:(qt + 1) * P, :], in_=t)

    nc.compile()
    return nc


def get_nc():
    if "nc" not in _CACHE:
        _CACHE["nc"] = _build()
    return _CACHE["nc"]


def make_in_maps(inputs):
    x = np.ascontiguousarray(np.asarray(inputs["x"], dtype=np.float32))
    ws = {
        k: np.ascontiguousarray(np.asarray(inputs[k], dtype=np.float32))
        for k in ("w_q", "w_k", "w_v", "w_o", "ln_gamma", "ln_beta")
    }
    in_maps = []
    for c in range(8):
        b, hf = c // 2, c % 2
        in_maps.append(
            {
                "x_kv": x[b],
                "x_q": np.ascontiguousarray(x[b, hf * NQ:(hf + 1) * NQ]),
                **ws,
            }
        )
    return in_maps


def gather(results):
    out = np.empty((B, S, D), dtype=np.float32)
    attn = np.empty((B, H, S, S), dtype=np.float32)
    for c in range(8):
        b, hf = c // 2, c % 2
        qsl = slice(hf * NQ, (hf + 1) * NQ)
        out[b, qsl] = results[c]["out_q"]
        at = results[c]["attn_t"].reshape(H, S, NQ)  # [h, kk, q]
        attn[b, :, qsl, :] = at.transpose(0, 2, 1)
    return out, attn


def kernel(**inputs):
    nc = get_nc()
    in_maps = make_in_maps(inputs)
    res = run_bass_kernel_spmd(nc, in_maps, core_ids=list(range(8)))
    return gather(res.results)


# revision 15
# speedup vs baseline: 1.0173x; 1.0173x over previous
"""Trainium2 Bass kernel for a post-LN MultiHeadAttention block.

Problem: x:(4,2048,1024), 16 heads x 64 dims, returns (out, attn) where
  out  = LayerNorm(ctx @ w_o + x) : (4, 2048, 1024)
  attn = softmax(q k^T / 8)       : (4, 16, 2048, 2048)

Sharding: 8 cores = 4 batches x 2 query-halves.  Each core computes
K/V for its whole batch (duplicated across the 2 half-cores) and
queries/attention/output for its 1024 query rows.  No collectives.

On-chip dataflow (all fp32, matmuls in float32r = full-rate fp32):
  - x is PE-transposed once into feature-major XT.
  - QT/KT feature-major via matmul(lhsT=W, rhs=XT); V seq-major via
    matmul(lhsT=XT, rhs=w_v) with a ones-column per head appended.
  - Scores are computed transposed, S^T[kk,q] (softmax along
    partitions); exp on ACT; the softmax denominator falls out of the
    PV matmul through the ones column (row 64 of the [65,q] psum).
  - attn is written to HBM transposed (h, kk, q); the host transposes
    while gathering.
  - Output proj seq-major via matmul(lhsT=ctxT, rhs=w_o), fused
    residual + LayerNorm (bn_stats/bn_aggr) on-chip.
"""

import sys

sys.path.insert(0, "/opt/trn_rl_repo")

from contextlib import ExitStack

import numpy as np

import concourse.bacc as bacc
import concourse.bass as bass
import concourse.tile as tile
from concourse import mybir
from concourse.alu_op_type import AluOpType
from concourse.bass_utils import run_bass_kernel_spmd
from concourse.masks import make_identity

FP32 = mybir.dt.float32
FP32R = mybir.dt.float32r
AF = mybir.ActivationFunctionType

B, S, D = 4, 2048, 1024
H, DK = 16, 64
NQ = S // 2          # query rows per core
P = 128
NF = D // P          # 8 feature/dmodel tiles
NS = S // P          # 16 seq tiles (kk)
NSQ = NQ // P        # 8 query seq tiles
QC = 512             # query chunk (matmul moving dim)
NQC = NQ // QC       # 2 query chunks
LN_EPS = 1e-6

_CACHE = {}


def r(ap):
    return ap if ap.dtype == FP32R else ap.bitcast(FP32R)


def _build():
    nc = bacc.Bacc("TRN2", target_bir_lowering=False, debug=False, num_devices=8)

    x_kv = nc.dram_tensor("x_kv", [S, D], FP32, kind="ExternalInput").ap()
    x_q = nc.dram_tensor("x_q", [NQ, D], FP32, kind="ExternalInput").ap()
    w_q = nc.dram_tensor("w_q", [D, D], FP32, kind="ExternalInput").ap()
    w_k = nc.dram_tensor("w_k", [D, D], FP32, kind="ExternalInput").ap()
    w_v = nc.dram_tensor("w_v", [D, D], FP32, kind="ExternalInput").ap()
    w_o = nc.dram_tensor("w_o", [D, D], FP32, kind="ExternalInput").ap()
    gam = nc.dram_tensor("ln_gamma", [D], FP32, kind="ExternalInput").ap()
    bet = nc.dram_tensor("ln_beta", [D], FP32, kind="ExternalInput").ap()

    # attn, stored transposed: [h, kk-tile, kk-within, q]
    attn_t = nc.dram_tensor("attn_t", [H, NS, P, NQ], FP32, kind="ExternalOutput").ap()
    out_q = nc.dram_tensor("out_q", [NQ, D], FP32, kind="ExternalOutput").ap()

    # DRAM scratch (per-core local)
    v_scr = nc.dram_tensor("v_scr", [H, 4, P, NS // 4, DK + 1], FP32R).ap()
    qt_scr = nc.dram_tensor("qt_scr", [NF, P, NQ], FP32R).ap()
    ctx_scr = nc.dram_tensor("ctx_scr", [NF, P, NQ], FP32R).ap()

    with ExitStack() as ctx:
        tc = ctx.enter_context(tile.TileContext(nc))

        const = ctx.enter_context(tc.tile_pool(name="const", bufs=1))
        ident = const.tile([P, P], FP32)
        make_identity(nc, ident)

        # KT stays in SBUF until the end of attention (outermost pool).
        kt_pool = ctx.enter_context(tc.tile_pool(name="kt", bufs=1))
        KT = [kt_pool.tile([P, S], FP32R, tag=f"kt{f}", name=f"KT{f}") for f in range(NF)]

        # ---- Phase A0q/A1: XTq via PE transpose, QT -> DRAM, free XTq ----
        with tc.tile_pool(name="xq_feat", bufs=1) as xtq_pool, \
             tc.tile_pool(name="xin", bufs=3) as xin_pool, \
             tc.tile_pool(name="tp_ps", bufs=4, space="PSUM") as tp_ps, \
             tc.tile_pool(name="wts", bufs=1) as w_pool, \
             tc.tile_pool(name="mm_ps", bufs=4, space="PSUM") as mm_ps, \
             tc.tile_pool(name="stage", bufs=3) as stage_pool:

            XTq = [xtq_pool.tile([P, NQ], FP32R, tag=f"xtq{f}", name=f"XTq{f}")
                   for f in range(NF)]
            for i in range(NSQ):
                xin = xin_pool.tile([P, D], FP32)
                nc.sync.dma_start(out=xin, in_=x_q[i * P:(i + 1) * P, :])
                for f in range(NF):
                    ps = tp_ps.tile([P, P], FP32)
                    nc.tensor.transpose(ps, xin[:, f * P:(f + 1) * P], ident)
                    nc.vector.tensor_copy(out=XTq[f][:, i * P:(i + 1) * P], in_=ps)

            wq_sb = w_pool.tile([P, NF, D], FP32R, tag="w")
            nc.sync.dma_start(out=wq_sb, in_=r(w_q.rearrange("(k p) f -> p k f", p=P)))
            for f in range(NF):
                qt_t = stage_pool.tile([P, NQ], FP32R, tag="qt_stage")
                for n in range(NQC):
                    ps = mm_ps.tile([P, QC], FP32)
                    for k in range(NF):
                        nc.tensor.matmul(
                            ps,
                            lhsT=r(wq_sb[:, k, f * P:(f + 1) * P]),
                            rhs=r(XTq[k][:, n * QC:(n + 1) * QC]),
                            start=(k == 0),
                            stop=(k == NF - 1),
                        )
                    nc.vector.tensor_copy(out=qt_t[:, n * QC:(n + 1) * QC], in_=ps)
                nc.sync.dma_start(out=qt_scr[f], in_=qt_t)

        # ---- Phases A0kv/A2/A3: XT, then KT (resident) and V -> DRAM ----
        with tc.tile_pool(name="xt", bufs=1) as xt_pool:
            XT = [xt_pool.tile([P, S], FP32R, tag=f"xt{f}", name=f"XT{f}")
                  for f in range(NF)]
            with tc.tile_pool(name="xin2", bufs=2) as xin_pool, \
                 tc.tile_pool(name="tp_ps2", bufs=4, space="PSUM") as tp_ps, \
                 tc.tile_pool(name="wts2", bufs=1) as w_pool, \
                 tc.tile_pool(name="mm_ps2", bufs=4, space="PSUM") as mm_ps, \
                 tc.tile_pool(name="stage2", bufs=3) as stage_pool:

                for i in range(NS):
                    xin = xin_pool.tile([P, D], FP32)
                    nc.sync.dma_start(out=xin, in_=x_kv[i * P:(i + 1) * P, :])
                    for f in range(NF):
                        ps = tp_ps.tile([P, P], FP32)
                        nc.tensor.transpose(ps, xin[:, f * P:(f + 1) * P], ident)
                        nc.vector.tensor_copy(out=XT[f][:, i * P:(i + 1) * P], in_=ps)

                wk_sb = w_pool.tile([P, NF, D], FP32R, tag="w")
                nc.sync.dma_start(out=wk_sb, in_=r(w_k.rearrange("(k p) f -> p k f", p=P)))
                for f in range(NF):
                    for n in range(S // QC):
                        ps = mm_ps.tile([P, QC], FP32)
                        for k in range(NF):
                            nc.tensor.matmul(
                                ps,
                                lhsT=r(wk_sb[:, k, f * P:(f + 1) * P]),
                                rhs=r(XT[k][:, n * QC:(n + 1) * QC]),
                                start=(k == 0),
                                stop=(k == NF - 1),
                            )
                        nc.vector.tensor_copy(out=KT[f][:, n * QC:(n + 1) * QC], in_=ps)

                wv_sb = w_pool.tile([P, NF, D], FP32R, tag="w")
                nc.sync.dma_start(out=wv_sb, in_=r(w_v.rearrange("(k p) f -> p k f", p=P)))
                NSH = NS // 4
                for half in range(4):
                    v_t = stage_pool.tile([P, NSH, H, DK + 1], FP32R, tag="v_stage", bufs=1)
                    for mm_ in range(NSH):
                        m = half * NSH + mm_
                        for n in range(2):
                            ps = mm_ps.tile([P, QC], FP32)
                            for k in range(NF):
                                nc.tensor.matmul(
                                    ps,
                                    lhsT=r(XT[k][:, m * P:(m + 1) * P]),
                                    rhs=r(wv_sb[:, k, n * QC:(n + 1) * QC]),
                                    start=(k == 0),
                                    stop=(k == NF - 1),
                                )
                            nc.vector.tensor_copy(
                                out=v_t[:, mm_, n * 8:(n + 1) * 8, 0:DK],
                                in_=ps.rearrange("p (h d) -> p h d", d=DK),
                            )
                    nc.vector.memset(v_t[:, :, :, DK:DK + 1].bitcast(FP32), 1.0)
                    for h in range(H):
                        nc.sync.dma_start(
                            out=v_scr[h, half], in_=v_t[:, :, h, :]
                        )

        # ---- Attention ----
        with tc.tile_pool(name="qt_pair", bufs=2) as qt_pool, \
             tc.tile_pool(name="vh", bufs=2) as vh_pool, \
             tc.tile_pool(name="slab", bufs=2) as slab_pool, \
             tc.tile_pool(name="recb", bufs=2) as recb_pool, \
             tc.tile_pool(name="ctxp", bufs=2) as ctxp_pool, \
             tc.tile_pool(name="st_ps", bufs=4, space="PSUM") as st_ps_pool, \
             tc.tile_pool(name="ctx_ps", bufs=2, space="PSUM") as ctx_ps_pool:

            for hp in range(H // 2):
                qt_p = qt_pool.tile([P, NQ], FP32R)
                nc.sync.dma_start(out=qt_p, in_=qt_scr[hp])
                ctx_pair = ctxp_pool.tile([P, NQ], FP32R)
                for hh in range(2):
                    h = 2 * hp + hh
                    r0 = DK * hh
                    v_h = vh_pool.tile([P, 4, NS // 4, DK + 1], FP32R)
                    nc.sync.dma_start(
                        out=v_h, in_=v_scr[h].rearrange("x p m c -> p x m c")
                    )
                    v_hf = v_h.rearrange("p x m c -> p (x m) c")
                    for qc in range(NQC):
                        slab = slab_pool.tile([P, NS, QC], FP32R)
                        ctx_ps = ctx_ps_pool.tile([DK + 1, QC], FP32)
                        for m in range(NS):
                            st_ps = st_ps_pool.tile([P, QC], FP32)
                            nc.tensor.matmul(
                                st_ps,
                                lhsT=r(KT[hp][r0:r0 + DK, m * P:(m + 1) * P]),
                                rhs=r(qt_p[r0:r0 + DK, qc * QC:(qc + 1) * QC]),
                                start=True,
                                stop=True,
                            )
                            nc.scalar.activation(
                                out=slab[:, m, :], in_=st_ps, func=AF.Exp, scale=0.125
                            )
                            nc.tensor.matmul(
                                ctx_ps,
                                lhsT=r(v_hf[:, m, :]),
                                rhs=r(slab[:, m, :]),
                                start=(m == 0),
                                stop=(m == NS - 1),
                            )
                        srow = recb_pool.tile([1, QC], FP32, tag="sr")
                        nc.scalar.copy(out=srow, in_=ctx_ps[DK:DK + 1, :])
                        sumB = recb_pool.tile([P, QC], FP32, tag="sb")
                        nc.gpsimd.partition_broadcast(sumB, srow)
                        recB = recb_pool.tile([P, QC], FP32, tag="rb")
                        nc.vector.reciprocal(out=recB, in_=sumB)
                        nc.vector.tensor_tensor(
                            out=ctx_pair[r0:r0 + DK, qc * QC:(qc + 1) * QC],
                            in0=ctx_ps[0:DK, :],
                            in1=recB[0:DK, :],
                            op=AluOpType.mult,
                        )
                        nc.vector.tensor_tensor(
                            out=slab,
                            in0=slab,
                            in1=recB.unsqueeze(1).broadcast_to((P, NS, QC)),
                            op=AluOpType.mult,
                        )
                        nc.sync.dma_start(
                            out=attn_t.rearrange("h m p q -> h p m q")[h][
                                :, :, qc * QC:(qc + 1) * QC
                            ],
                            in_=slab.bitcast(FP32),
                        )
                nc.sync.dma_start(out=ctx_scr[hp], in_=ctx_pair)

        # ---- Output projection + residual + LayerNorm ----
        with tc.tile_pool(name="wts3", bufs=1) as w_pool, \
             tc.tile_pool(name="mm_ps3", bufs=4, space="PSUM") as mm_ps, \
             tc.tile_pool(name="ctxo", bufs=3) as ctxo_pool, \
             tc.tile_pool(name="y", bufs=3) as y_pool, \
             tc.tile_pool(name="xr", bufs=3) as xr_pool, \
             tc.tile_pool(name="lnst", bufs=4) as ln_pool:

            gamB = const.tile([P, D], FP32)
            nc.gpsimd.dma_start(
                out=gamB,
                in_=bass.AP(tensor=gam.tensor, offset=gam.offset, ap=[[0, P], [1, D]]),
            )
            betB = const.tile([P, D], FP32)
            nc.gpsimd.dma_start(
                out=betB,
                in_=bass.AP(tensor=bet.tensor, offset=bet.offset, ap=[[0, P], [1, D]]),
            )
            eps_t = const.tile([P, 1], FP32)
            nc.vector.memset(eps_t, LN_EPS)

            wo_sb = w_pool.tile([P, NF, D], FP32R, tag="w")
            nc.sync.dma_start(out=wo_sb, in_=r(w_o.rearrange("(k p) f -> p k f", p=P)))

            for qt in range(NSQ):
                ctxo = ctxo_pool.tile([P, NF, P], FP32R)
                nc.sync.dma_start(
                    out=ctxo,
                    in_=ctx_scr[:, :, qt * P:(qt + 1) * P].rearrange("k p q -> p k q"),
                )
                y = y_pool.tile([P, D], FP32)
                xr = xr_pool.tile([P, D], FP32)
                nc.sync.dma_start(out=xr, in_=x_q[qt * P:(qt + 1) * P, :])
                for n in range(2):
                    ps = mm_ps.tile([P, QC], FP32)
                    for k in range(NF):
                        nc.tensor.matmul(
                            ps,
                            lhsT=r(ctxo[:, k, :]),
                            rhs=r(wo_sb[:, k, n * QC:(n + 1) * QC]),
                            start=(k == 0),
                            stop=(k == NF - 1),
                        )
                    nc.vector.tensor_tensor(
                        out=y[:, n * QC:(n + 1) * QC],
                        in0=ps,
                        in1=xr[:, n * QC:(n + 1) * QC],
                        op=AluOpType.add,
                    )
                stats = ln_pool.tile([P, 2, nc.vector.BN_STATS_DIM], FP32, tag="stats")
                for g in range(2):
                    nc.vector.bn_stats(out=stats[:, g, :], in_=y[:, g * 512:(g + 1) * 512])
                mv = ln_pool.tile([P, nc.vector.BN_AGGR_DIM], FP32, tag="mv")
                nc.vector.bn_aggr(out=mv, in_=stats)
                sd = ln_pool.tile([P, 1], FP32, tag="sd")
                nc.scalar.activation(
                    out=sd, in_=mv[:, 1:2], func=AF.Sqrt, bias=eps_t, scale=1.0
                )
                rstd = ln_pool.tile([P, 1], FP32, tag="rstd")
                nc.vector.reciprocal(out=rstd, in_=sd)
                t = y_pool.tile([P, D], FP32, tag="t")
                nc.vector.tensor_scalar(
                    out=t,
                    in0=y,
                    scalar1=mv[:, 0:1],
                    scalar2=rstd,
                    op0=AluOpType.subtract,
                    op1=AluOpType.mult,
                )
                nc.vector.tensor_tensor(out=t, in0=t, in1=gamB, op=AluOpType.mult)
                nc.vector.tensor_tensor(out=t, in0=t, in1=betB, op=AluOpType.add)
                nc.sync.dma_start(out=out_q[qt * P:(qt + 1) * P, :], in_=t)

    nc.compile()
    return nc


def get_nc():
    if "nc" not in _CACHE:
        _CACHE["nc"] = _build()
    return _CACHE["nc"]


def make_in_maps(inputs):
    x = np.ascontiguousarray(np.asarray(inputs["x"], dtype=np.float32))
    ws = {
        k: np.ascontiguousarray(np.asarray(inputs[k], dtype=np.float32))
        for k in ("w_q", "w_k", "w_v", "w_o", "ln_gamma", "ln_beta")
    }
    in_maps = []
    for c in range(8):
        b, hf = c // 2, c % 2
        in_maps.append(
            {
                "x_kv": x[b],
                "x_q": np.ascontiguousarray(x[b, hf * NQ:(hf + 1) * NQ]),
                **ws,
            }
        )
    return in_maps


def gather(results):
    out = np.empty((B, S, D), dtype=np.float32)
    attn = np.empty((B, H, S, S), dtype=np.float32)
    for c in range(8):
        b, hf = c // 2, c % 2
        qsl = slice(hf * NQ, (hf + 1) * NQ)
        out[b, qsl] = results[c]["out_q"]
        at = results[c]["attn_t"].reshape(H, S, NQ)  # [h, kk, q]
        attn[b, :, qsl, :] = at.transpose(0, 2, 1)
    return out, attn


def kernel(**inputs):
    nc = get_nc()
    in_maps = make_in_maps(inputs)
    res = run_bass_kernel_spmd(nc, in_maps, core_ids=list(range(8)))
    return gather(res.results)


# revision 16
# speedup vs baseline: 1.0987x; 1.0800x over previous
"""Trainium2 Bass kernel for a post-LN MultiHeadAttention block.

Problem: x:(4,2048,1024), 16 heads x 64 dims, returns (out, attn) where
  out  = LayerNorm(ctx @ w_o + x) : (4, 2048, 1024)
  attn = softmax(q k^T / 8)       : (4, 16, 2048, 2048)

Sharding: 8 cores = 4 batches x 2 query-halves.  Each core computes
K/V for its whole batch (duplicated across the 2 half-cores) and
queries/attention/output for its 1024 query rows.  No collectives.

On-chip dataflow (all fp32, matmuls in float32r = full-rate fp32):
  - x is PE-transposed once into feature-major XT.
  - QT/KT feature-major via matmul(lhsT=W, rhs=XT); V seq-major via
    matmul(lhsT=XT, rhs=w_v) with a ones-column per head appended.
  - Scores are computed transposed, S^T[kk,q] (softmax along
    partitions); exp on ACT; the softmax denominator falls out of the
    PV matmul through the ones column (row 64 of the [65,q] psum).
  - attn is written to HBM transposed (h, kk, q); the host transposes
    while gathering.
  - Output proj seq-major via matmul(lhsT=ctxT, rhs=w_o), fused
    residual + LayerNorm (bn_stats/bn_aggr) on-chip.
"""

import sys

sys.path.insert(0, "/opt/trn_rl_repo")

from contextlib import ExitStack

import numpy as np

import concourse.bacc as bacc
import concourse.bass as bass
import concourse.tile as tile
from concourse import mybir
from concourse.alu_op_type import AluOpType
from concourse.bass_utils import run_bass_kernel_spmd
from concourse.masks import make_identity

FP32 = mybir.dt.float32
FP32R = mybir.dt.float32r
AF = mybir.ActivationFunctionType

B, S, D = 4, 2048, 1024
H, DK = 16, 64
NQ = S // 2          # query rows per core
P = 128
NF = D // P          # 8 feature/dmodel tiles
NS = S // P          # 16 seq tiles (kk)
NSQ = NQ // P        # 8 query seq tiles
QC = 512             # query chunk (matmul moving dim)
NQC = NQ // QC       # 2 query chunks
LN_EPS = 1e-6

_CACHE = {}


def r(ap):
    return ap if ap.dtype == FP32R else ap.bitcast(FP32R)


def _build():
    nc = bacc.Bacc("TRN2", target_bir_lowering=False, debug=False, num_devices=8)

    x_kv = nc.dram_tensor("x_kv", [S, D], FP32, kind="ExternalInput").ap()
    x_q = nc.dram_tensor("x_q", [NQ, D], FP32, kind="ExternalInput").ap()
    w_q = nc.dram_tensor("w_q", [D, D], FP32, kind="ExternalInput").ap()
    w_k = nc.dram_tensor("w_k", [D, D], FP32, kind="ExternalInput").ap()
    w_v = nc.dram_tensor("w_v", [D, D], FP32, kind="ExternalInput").ap()
    w_o = nc.dram_tensor("w_o", [D, D], FP32, kind="ExternalInput").ap()
    gam = nc.dram_tensor("ln_gamma", [D], FP32, kind="ExternalInput").ap()
    bet = nc.dram_tensor("ln_beta", [D], FP32, kind="ExternalInput").ap()

    # attn, stored transposed: [h, kk-tile, kk-within, q]
    attn_t = nc.dram_tensor("attn_t", [H, NS, P, NQ], FP32, kind="ExternalOutput").ap()
    out_q = nc.dram_tensor("out_q", [NQ, D], FP32, kind="ExternalOutput").ap()

    # DRAM scratch (per-core local)
    v_scr = nc.dram_tensor("v_scr", [H, 4, P, NS // 4, DK + 1], FP32R).ap()
    qt_scr = nc.dram_tensor("qt_scr", [NF, P, NQ], FP32R).ap()
    ctx_scr = nc.dram_tensor("ctx_scr", [NF, P, NQ], FP32R).ap()

    with ExitStack() as ctx:
        tc = ctx.enter_context(tile.TileContext(nc))

        const = ctx.enter_context(tc.tile_pool(name="const", bufs=1))
        ident = const.tile([P, P], FP32)
        make_identity(nc, ident)

        # KT stays in SBUF until the end of attention (outermost pool).
        kt_pool = ctx.enter_context(tc.tile_pool(name="kt", bufs=1))
        KT = [kt_pool.tile([P, S], FP32R, tag=f"kt{f}", name=f"KT{f}") for f in range(NF)]

        # ---- Phase A0q/A1: XTq via PE transpose, QT -> DRAM, free XTq ----
        with tc.tile_pool(name="xq_feat", bufs=1) as xtq_pool, \
             tc.tile_pool(name="xin", bufs=3) as xin_pool, \
             tc.tile_pool(name="tp_ps", bufs=4, space="PSUM") as tp_ps, \
             tc.tile_pool(name="wts", bufs=1) as w_pool, \
             tc.tile_pool(name="mm_ps", bufs=4, space="PSUM") as mm_ps, \
             tc.tile_pool(name="stage", bufs=3) as stage_pool:

            XTq = [xtq_pool.tile([P, NQ], FP32R, tag=f"xtq{f}", name=f"XTq{f}")
                   for f in range(NF)]
            for i in range(NSQ):
                xin = xin_pool.tile([P, D], FP32)
                nc.sync.dma_start(out=xin, in_=x_q[i * P:(i + 1) * P, :])
                for f in range(NF):
                    ps = tp_ps.tile([P, P], FP32)
                    nc.tensor.transpose(ps, xin[:, f * P:(f + 1) * P], ident)
                    nc.vector.tensor_copy(out=XTq[f][:, i * P:(i + 1) * P], in_=ps)

            wq_sb = w_pool.tile([P, NF, D], FP32R, tag="w")
            nc.sync.dma_start(out=wq_sb, in_=r(w_q.rearrange("(k p) f -> p k f", p=P)))
            for f in range(NF):
                qt_t = stage_pool.tile([P, NQ], FP32R, tag="qt_stage")
                for n in range(NQC):
                    ps = mm_ps.tile([P, QC], FP32)
                    for k in range(NF):
                        nc.tensor.matmul(
                            ps,
                            lhsT=r(wq_sb[:, k, f * P:(f + 1) * P]),
                            rhs=r(XTq[k][:, n * QC:(n + 1) * QC]),
                            start=(k == 0),
                            stop=(k == NF - 1),
                        )
                    nc.vector.tensor_copy(out=qt_t[:, n * QC:(n + 1) * QC], in_=ps)
                nc.sync.dma_start(out=qt_scr[f], in_=qt_t)

        # ---- Phases A0kv/A2/A3: XT, then KT (resident) and V -> DRAM ----
        with tc.tile_pool(name="xt", bufs=1) as xt_pool:
            XT = [xt_pool.tile([P, S], FP32R, tag=f"xt{f}", name=f"XT{f}")
                  for f in range(NF)]
            with tc.tile_pool(name="xin2", bufs=2) as xin_pool, \
                 tc.tile_pool(name="tp_ps2", bufs=4, space="PSUM") as tp_ps, \
                 tc.tile_pool(name="wts2", bufs=1) as w_pool, \
                 tc.tile_pool(name="mm_ps2", bufs=4, space="PSUM") as mm_ps, \
                 tc.tile_pool(name="stage2", bufs=3) as stage_pool:

                for i in range(NS):
                    xin = xin_pool.tile([P, D], FP32)
                    nc.sync.dma_start(out=xin, in_=x_kv[i * P:(i + 1) * P, :])
                    for f in range(NF):
                        ps = tp_ps.tile([P, P], FP32)
                        nc.tensor.transpose(ps, xin[:, f * P:(f + 1) * P], ident)
                        nc.vector.tensor_copy(out=XT[f][:, i * P:(i + 1) * P], in_=ps)

                wk_sb = w_pool.tile([P, NF, D], FP32R, tag="w")
                nc.sync.dma_start(out=wk_sb, in_=r(w_k.rearrange("(k p) f -> p k f", p=P)))
                for f in range(NF):
                    for n in range(S // QC):
                        ps = mm_ps.tile([P, QC], FP32)
                        for k in range(NF):
                            nc.tensor.matmul(
                                ps,
                                lhsT=r(wk_sb[:, k, f * P:(f + 1) * P]),
                                rhs=r(XT[k][:, n * QC:(n + 1) * QC]),
                                start=(k == 0),
                                stop=(k == NF - 1),
                            )
                        nc.vector.tensor_copy(out=KT[f][:, n * QC:(n + 1) * QC], in_=ps)

                wv_sb = w_pool.tile([P, NF, D], FP32R, tag="w")
                nc.sync.dma_start(out=wv_sb, in_=r(w_v.rearrange("(k p) f -> p k f", p=P)))
                NSH = NS // 4
                for half in range(4):
                    v_t = stage_pool.tile([P, NSH, H, DK + 1], FP32R, tag="v_stage", bufs=1)
                    for mm_ in range(NSH):
                        m = half * NSH + mm_
                        for n in range(2):
                            ps = mm_ps.tile([P, QC], FP32)
                            for k in range(NF):
                                nc.tensor.matmul(
                                    ps,
                                    lhsT=r(XT[k][:, m * P:(m + 1) * P]),
                                    rhs=r(wv_sb[:, k, n * QC:(n + 1) * QC]),
                                    start=(k == 0),
                                    stop=(k == NF - 1),
                                )
                            nc.vector.tensor_copy(
                                out=v_t[:, mm_, n * 8:(n + 1) * 8, 0:DK],
                                in_=ps.rearrange("p (h d) -> p h d", d=DK),
                            )
                    nc.vector.memset(v_t[:, :, :, DK:DK + 1].bitcast(FP32), 1.0)
                    for h in range(H):
                        nc.sync.dma_start(
                            out=v_scr[h, half], in_=v_t[:, :, h, :]
                        )

        # ---- Attention ----
        with tc.tile_pool(name="qt_pair", bufs=2) as qt_pool, \
             tc.tile_pool(name="vh", bufs=2) as vh_pool, \
             tc.tile_pool(name="slab", bufs=3) as slab_pool, \
             tc.tile_pool(name="recb", bufs=2) as recb_pool, \
             tc.tile_pool(name="ctxp", bufs=2) as ctxp_pool, \
             tc.tile_pool(name="st_ps", bufs=4, space="PSUM") as st_ps_pool, \
             tc.tile_pool(name="ctx_ps", bufs=2, space="PSUM") as ctx_ps_pool:

            for hp in range(H // 2):
                qt_p = qt_pool.tile([P, NQ], FP32R)
                nc.sync.dma_start(out=qt_p, in_=qt_scr[hp])
                ctx_pair = ctxp_pool.tile([P, NQ], FP32R)
                for hh in range(2):
                    h = 2 * hp + hh
                    r0 = DK * hh
                    v_h = vh_pool.tile([P, 4, NS // 4, DK + 1], FP32R)
                    nc.sync.dma_start(
                        out=v_h, in_=v_scr[h].rearrange("x p m c -> p x m c")
                    )
                    v_hf = v_h.rearrange("p x m c -> p (x m) c")
                    for qc in range(NQC):
                        slab = slab_pool.tile([P, NS, QC], FP32R)
                        ctx_ps = ctx_ps_pool.tile([DK + 1, QC], FP32)
                        for m in range(NS):
                            st_ps = st_ps_pool.tile([P, QC], FP32)
                            nc.tensor.matmul(
                                st_ps,
                                lhsT=r(KT[hp][r0:r0 + DK, m * P:(m + 1) * P]),
                                rhs=r(qt_p[r0:r0 + DK, qc * QC:(qc + 1) * QC]),
                                start=True,
                                stop=True,
                            )
                            nc.scalar.activation(
                                out=slab[:, m, :], in_=st_ps, func=AF.Exp, scale=0.125
                            )
                            nc.tensor.matmul(
                                ctx_ps,
                                lhsT=r(v_hf[:, m, :]),
                                rhs=r(slab[:, m, :]),
                                start=(m == 0),
                                stop=(m == NS - 1),
                            )
                        srow = recb_pool.tile([1, QC], FP32, tag="sr")
                        nc.vector.tensor_copy(out=srow, in_=ctx_ps[DK:DK + 1, :])
                        sumB = recb_pool.tile([P, QC], FP32, tag="sb")
                        nc.gpsimd.partition_broadcast(sumB, srow)
                        recB = recb_pool.tile([P, QC], FP32, tag="rb")
                        nc.vector.reciprocal(out=recB, in_=sumB)
                        nc.vector.tensor_tensor(
                            out=ctx_pair[r0:r0 + DK, qc * QC:(qc + 1) * QC],
                            in0=ctx_ps[0:DK, :],
                            in1=recB[0:DK, :],
                            op=AluOpType.mult,
                        )
                        nc.vector.tensor_tensor(
                            out=slab,
                            in0=slab,
                            in1=recB.unsqueeze(1).broadcast_to((P, NS, QC)),
                            op=AluOpType.mult,
                        )
                        nc.sync.dma_start(
                            out=attn_t.rearrange("h m p q -> h p m q")[h][
                                :, :, qc * QC:(qc + 1) * QC
                            ],
                            in_=slab.bitcast(FP32),
                        )
                nc.sync.dma_start(out=ctx_scr[hp], in_=ctx_pair)

        # ---- Output projection + residual + LayerNorm ----
        with tc.tile_pool(name="wts3", bufs=1) as w_pool, \
             tc.tile_pool(name="mm_ps3", bufs=4, space="PSUM") as mm_ps, \
             tc.tile_pool(name="ctxo", bufs=3) as ctxo_pool, \
             tc.tile_pool(name="y", bufs=3) as y_pool, \
             tc.tile_pool(name="xr", bufs=3) as xr_pool, \
             tc.tile_pool(name="lnst", bufs=4) as ln_pool:

            gamB = const.tile([P, D], FP32)
            nc.gpsimd.dma_start(
                out=gamB,
                in_=bass.AP(tensor=gam.tensor, offset=gam.offset, ap=[[0, P], [1, D]]),
            )
            betB = const.tile([P, D], FP32)
            nc.gpsimd.dma_start(
                out=betB,
                in_=bass.AP(tensor=bet.tensor, offset=bet.offset, ap=[[0, P], [1, D]]),
            )
            eps_t = const.tile([P, 1], FP32)
            nc.vector.memset(eps_t, LN_EPS)

            wo_sb = w_pool.tile([P, NF, D], FP32R, tag="w")
            nc.sync.dma_start(out=wo_sb, in_=r(w_o.rearrange("(k p) f -> p k f", p=P)))

            for qt in range(NSQ):
                ctxo = ctxo_pool.tile([P, NF, P], FP32R)
                nc.sync.dma_start(
                    out=ctxo,
                    in_=ctx_scr[:, :, qt * P:(qt + 1) * P].rearrange("k p q -> p k q"),
                )
                y = y_pool.tile([P, D], FP32)
                xr = xr_pool.tile([P, D], FP32)
                nc.sync.dma_start(out=xr, in_=x_q[qt * P:(qt + 1) * P, :])
                for n in range(2):
                    ps = mm_ps.tile([P, QC], FP32)
                    for k in range(NF):
                        nc.tensor.matmul(
                            ps,
                            lhsT=r(ctxo[:, k, :]),
                            rhs=r(wo_sb[:, k, n * QC:(n + 1) * QC]),
                            start=(k == 0),
                            stop=(k == NF - 1),
                        )
                    nc.vector.tensor_tensor(
                        out=y[:, n * QC:(n + 1) * QC],
                        in0=ps,
                        in1=xr[:, n * QC:(n + 1) * QC],
                        op=AluOpType.add,
                    )
                stats = ln_pool.tile([P, 2, nc.vector.BN_STATS_DIM], FP32, tag="stats")
                for g in range(2):
                    nc.vector.bn_stats(out=stats[:, g, :], in_=y[:, g * 512:(g + 1) * 512])
                mv = ln_pool.tile([P, nc.vector.BN_AGGR_DIM], FP32, tag="mv")
                nc.vector.bn_aggr(out=mv, in_=stats)
                sd = ln_pool.tile([P, 1], FP32, tag="sd")
                nc.scalar.activation(
                    out=sd, in_=mv[:, 1:2], func=AF.Sqrt, bias=eps_t, scale=1.0
                )
                rstd = ln_pool.tile([P, 1], FP32, tag="rstd")
                nc.vector.reciprocal(out=rstd, in_=sd)
                t = y_pool.tile([P, D], FP32, tag="t")
                nc.vector.tensor_scalar(
                    out=t,
                    in0=y,
                    scalar1=mv[:, 0:1],
                    scalar2=rstd,
                    op0=AluOpType.subtract,
                    op1=AluOpType.mult,
                )
                nc.vector.tensor_tensor(out=t, in0=t, in1=gamB, op=AluOpType.mult)
                nc.vector.tensor_tensor(out=t, in0=t, in1=betB, op=AluOpType.add)
                nc.sync.dma_start(out=out_q[qt * P:(qt + 1) * P, :], in_=t)

    nc.compile()
    return nc


def get_nc():
    if "nc" not in _CACHE:
        _CACHE["nc"] = _build()
    return _CACHE["nc"]


def make_in_maps(inputs):
    x = np.ascontiguousarray(np.asarray(inputs["x"], dtype=np.float32))
    ws = {
        k: np.ascontiguousarray(np.asarray(inputs[k], dtype=np.float32))
        for k in ("w_q", "w_k", "w_v", "w_o", "ln_gamma", "ln_beta")
    }
    in_maps = []
    for c in range(8):
        b, hf = c // 2, c % 2
        in_maps.append(
            {
                "x_kv": x[b],
                "x_q": np.ascontiguousarray(x[b, hf * NQ:(hf + 1) * NQ]),
                **ws,
            }
        )
    return in_maps


def gather(results):
    out = np.empty((B, S, D), dtype=np.float32)
    attn = np.empty((B, H, S, S), dtype=np.float32)
    for c in range(8):
        b, hf = c // 2, c % 2
        qsl = slice(hf * NQ, (hf + 1) * NQ)
        out[b, qsl] = results[c]["out_q"]
        at = results[c]["attn_t"].reshape(H, S, NQ)  # [h, kk, q]
        attn[b, :, qsl, :] = at.transpose(0, 2, 1)
    return out, attn


def kernel(**inputs):
    nc = get_nc()
    in_maps = make_in_maps(inputs)
    res = run_bass_kernel_spmd(nc, in_maps, core_ids=list(range(8)))
    return gather(res.results)


# revision 17
# speedup vs baseline: 1.3226x; 1.2039x over previous
"""Trainium2 Bass kernel for a post-LN MultiHeadAttention block.

Problem: x:(4,2048,1024), 16 heads x 64 dims, returns (out, attn) where
  out  = LayerNorm(ctx @ w_o + x) : (4, 2048, 1024)
  attn = softmax(q k^T / 8)       : (4, 16, 2048, 2048)

Sharding: 8 cores = 4 batches x 2 query-halves.  Each core computes
K/V for its whole batch (duplicated across the 2 half-cores) and
queries/attention/output for its 1024 query rows.  No collectives.

On-chip dataflow (all fp32, matmuls in float32r = full-rate fp32):
  - x is PE-transposed once into feature-major XT.
  - QT/KT feature-major via matmul(lhsT=W, rhs=XT); V seq-major via
    matmul(lhsT=XT, rhs=w_v) with a ones-column per head appended.
  - Scores are computed transposed, S^T[kk,q] (softmax along
    partitions); exp on ACT; the softmax denominator falls out of the
    PV matmul through the ones column (row 64 of the [65,q] psum).
  - attn is written to HBM transposed (h, kk, q); the host transposes
    while gathering.
  - Output proj seq-major via matmul(lhsT=ctxT, rhs=w_o), fused
    residual + LayerNorm (bn_stats/bn_aggr) on-chip.
"""

import sys

sys.path.insert(0, "/opt/trn_rl_repo")

from contextlib import ExitStack

import numpy as np

import concourse.bacc as bacc
import concourse.bass as bass
import concourse.tile as tile
from concourse import mybir
from concourse.alu_op_type import AluOpType
from concourse.bass_utils import run_bass_kernel_spmd
from concourse.masks import make_identity

FP32 = mybir.dt.float32
FP32R = mybir.dt.float32r
AF = mybir.ActivationFunctionType

B, S, D = 4, 2048, 1024
H, DK = 16, 64
NQ = S // 2          # query rows per core
P = 128
NF = D // P          # 8 feature/dmodel tiles
NS = S // P          # 16 seq tiles (kk)
NSQ = NQ // P        # 8 query seq tiles
QC = 512             # query chunk (matmul moving dim)
NQC = NQ // QC       # 2 query chunks
LN_EPS = 1e-6

_CACHE = {}


def r(ap):
    return ap if ap.dtype == FP32R else ap.bitcast(FP32R)


def _build():
    nc = bacc.Bacc("TRN2", target_bir_lowering=False, debug=False, num_devices=8)

    x_kv = nc.dram_tensor("x_kv", [S, D], FP32, kind="ExternalInput").ap()
    x_q = nc.dram_tensor("x_q", [NQ, D], FP32, kind="ExternalInput").ap()
    w_q = nc.dram_tensor("w_q", [D, D], FP32, kind="ExternalInput").ap()
    w_k = nc.dram_tensor("w_k", [D, D], FP32, kind="ExternalInput").ap()
    w_v = nc.dram_tensor("w_v", [D, D], FP32, kind="ExternalInput").ap()
    w_o = nc.dram_tensor("w_o", [D, D], FP32, kind="ExternalInput").ap()
    gam = nc.dram_tensor("ln_gamma", [D], FP32, kind="ExternalInput").ap()
    bet = nc.dram_tensor("ln_beta", [D], FP32, kind="ExternalInput").ap()

    # attn, stored transposed: [h, kk-tile, kk-within, q]
    attn_t = nc.dram_tensor("attn_t", [H, NS, P, NQ], FP32, kind="ExternalOutput").ap()
    out_q = nc.dram_tensor("out_q", [NQ, D], FP32, kind="ExternalOutput").ap()

    # DRAM scratch (per-core local)
    v_scr = nc.dram_tensor("v_scr", [NS, P, H, DK + 1], FP32R).ap()
    qt_scr = nc.dram_tensor("qt_scr", [NF, P, NQ], FP32R).ap()
    ctx_scr = nc.dram_tensor("ctx_scr", [NF, P, NQ], FP32R).ap()

    with ExitStack() as ctx:
        tc = ctx.enter_context(tile.TileContext(nc))

        const = ctx.enter_context(tc.tile_pool(name="const", bufs=1))
        ident = const.tile([P, P], FP32)
        make_identity(nc, ident)

        # KT stays in SBUF until the end of attention (outermost pool).
        kt_pool = ctx.enter_context(tc.tile_pool(name="kt", bufs=1))
        KT = [kt_pool.tile([P, S], FP32R, tag=f"kt{f}", name=f"KT{f}") for f in range(NF)]

        # ---- Phase A0q/A1: XTq via PE transpose, QT -> DRAM, free XTq ----
        with tc.tile_pool(name="xq_feat", bufs=1) as xtq_pool, \
             tc.tile_pool(name="xin", bufs=3) as xin_pool, \
             tc.tile_pool(name="tp_ps", bufs=4, space="PSUM") as tp_ps, \
             tc.tile_pool(name="wts", bufs=1) as w_pool, \
             tc.tile_pool(name="mm_ps", bufs=4, space="PSUM") as mm_ps, \
             tc.tile_pool(name="stage", bufs=3) as stage_pool:

            XTq = [xtq_pool.tile([P, NQ], FP32R, tag=f"xtq{f}", name=f"XTq{f}")
                   for f in range(NF)]
            for i in range(NSQ):
                xin = xin_pool.tile([P, D], FP32)
                nc.scalar.dma_start(out=xin, in_=x_q[i * P:(i + 1) * P, :])
                for f in range(NF):
                    ps = tp_ps.tile([P, P], FP32)
                    nc.tensor.transpose(ps, xin[:, f * P:(f + 1) * P], ident)
                    nc.vector.tensor_copy(out=XTq[f][:, i * P:(i + 1) * P], in_=ps)

            wq_sb = w_pool.tile([P, NF, D], FP32R, tag="w")
            nc.scalar.dma_start(out=wq_sb, in_=r(w_q.rearrange("(k p) f -> p k f", p=P)))
            for f in range(NF):
                qt_t = stage_pool.tile([P, NQ], FP32R, tag="qt_stage")
                for n in range(NQC):
                    ps = mm_ps.tile([P, QC], FP32)
                    for k in range(NF):
                        nc.tensor.matmul(
                            ps,
                            lhsT=r(wq_sb[:, k, f * P:(f + 1) * P]),
                            rhs=r(XTq[k][:, n * QC:(n + 1) * QC]),
                            start=(k == 0),
                            stop=(k == NF - 1),
                        )
                    nc.vector.tensor_copy(out=qt_t[:, n * QC:(n + 1) * QC], in_=ps)
                nc.gpsimd.dma_start(out=qt_scr[f], in_=qt_t)

        # ---- Phases A0kv/A2/A3: XT, then KT (resident) and V -> DRAM ----
        with tc.tile_pool(name="xt", bufs=1) as xt_pool:
            XT = [xt_pool.tile([P, S], FP32R, tag=f"xt{f}", name=f"XT{f}")
                  for f in range(NF)]
            with tc.tile_pool(name="xin2", bufs=2) as xin_pool, \
                 tc.tile_pool(name="tp_ps2", bufs=4, space="PSUM") as tp_ps, \
                 tc.tile_pool(name="wts2", bufs=1) as w_pool, \
                 tc.tile_pool(name="mm_ps2", bufs=4, space="PSUM") as mm_ps, \
                 tc.tile_pool(name="stage2", bufs=3) as stage_pool:

                for i in range(NS):
                    xin = xin_pool.tile([P, D], FP32)
                    nc.scalar.dma_start(out=xin, in_=x_kv[i * P:(i + 1) * P, :])
                    for f in range(NF):
                        ps = tp_ps.tile([P, P], FP32)
                        nc.tensor.transpose(ps, xin[:, f * P:(f + 1) * P], ident)
                        nc.vector.tensor_copy(out=XT[f][:, i * P:(i + 1) * P], in_=ps)

                wk_sb = w_pool.tile([P, NF, D], FP32R, tag="w")
                nc.scalar.dma_start(out=wk_sb, in_=r(w_k.rearrange("(k p) f -> p k f", p=P)))
                for f in range(NF):
                    for n in range(S // QC):
                        ps = mm_ps.tile([P, QC], FP32)
                        for k in range(NF):
                            nc.tensor.matmul(
                                ps,
                                lhsT=r(wk_sb[:, k, f * P:(f + 1) * P]),
                                rhs=r(XT[k][:, n * QC:(n + 1) * QC]),
                                start=(k == 0),
                                stop=(k == NF - 1),
                            )
                        nc.vector.tensor_copy(out=KT[f][:, n * QC:(n + 1) * QC], in_=ps)

                wv_sb = w_pool.tile([P, NF, D], FP32R, tag="w")
                nc.scalar.dma_start(out=wv_sb, in_=r(w_v.rearrange("(k p) f -> p k f", p=P)))
                for m in range(NS):
                    v_t = stage_pool.tile([P, H, DK + 1], FP32R, tag="v_stage")
                    for n in range(2):
                        ps = mm_ps.tile([P, QC], FP32)
                        for k in range(NF):
                            nc.tensor.matmul(
                                ps,
                                lhsT=r(XT[k][:, m * P:(m + 1) * P]),
                                rhs=r(wv_sb[:, k, n * QC:(n + 1) * QC]),
                                start=(k == 0),
                                stop=(k == NF - 1),
                            )
                        nc.vector.tensor_copy(
                            out=v_t[:, n * 8:(n + 1) * 8, 0:DK],
                            in_=ps.rearrange("p (h d) -> p h d", d=DK),
                        )
                    nc.vector.memset(v_t[:, :, DK:DK + 1].bitcast(FP32), 1.0)
                    nc.gpsimd.dma_start(out=v_scr[m], in_=v_t)

        # ---- Attention ----
        with tc.tile_pool(name="qt_pair", bufs=2) as qt_pool, \
             tc.tile_pool(name="vh", bufs=2) as vh_pool, \
             tc.tile_pool(name="slab", bufs=3) as slab_pool, \
             tc.tile_pool(name="recb", bufs=2) as recb_pool, \
             tc.tile_pool(name="ctxp", bufs=2) as ctxp_pool, \
             tc.tile_pool(name="st_ps", bufs=4, space="PSUM") as st_ps_pool, \
             tc.tile_pool(name="ctx_ps", bufs=2, space="PSUM") as ctx_ps_pool:

            for hp in range(H // 2):
                qt_p = qt_pool.tile([P, NQ], FP32R)
                nc.scalar.dma_start(out=qt_p, in_=qt_scr[hp])
                ctx_pair = ctxp_pool.tile([P, NQ], FP32R)
                for hh in range(2):
                    h = 2 * hp + hh
                    r0 = DK * hh
                    v_h = vh_pool.tile([P, NS, DK + 1], FP32R)
                    nc.scalar.dma_start(
                        out=v_h, in_=v_scr[:, :, h, :].rearrange("m p c -> p m c")
                    )
                    v_hf = v_h
                    for qc in range(NQC):
                        slab = slab_pool.tile([P, NS, QC], FP32R)
                        ctx_ps = ctx_ps_pool.tile([DK + 1, QC], FP32)
                        for m in range(NS):
                            st_ps = st_ps_pool.tile([P, QC], FP32)
                            nc.tensor.matmul(
                                st_ps,
                                lhsT=r(KT[hp][r0:r0 + DK, m * P:(m + 1) * P]),
                                rhs=r(qt_p[r0:r0 + DK, qc * QC:(qc + 1) * QC]),
                                start=True,
                                stop=True,
                            )
                            nc.scalar.activation(
                                out=slab[:, m, :], in_=st_ps, func=AF.Exp, scale=0.125
                            )
                            nc.tensor.matmul(
                                ctx_ps,
                                lhsT=r(v_hf[:, m, :]),
                                rhs=r(slab[:, m, :]),
                                start=(m == 0),
                                stop=(m == NS - 1),
                            )
                        srow = recb_pool.tile([1, QC], FP32, tag="sr")
                        nc.vector.tensor_copy(out=srow, in_=ctx_ps[DK:DK + 1, :])
                        sumB = recb_pool.tile([P, QC], FP32, tag="sb")
                        nc.gpsimd.partition_broadcast(sumB, srow)
                        recB = recb_pool.tile([P, QC], FP32, tag="rb")
                        nc.vector.reciprocal(out=recB, in_=sumB)
                        nc.vector.tensor_tensor(
                            out=ctx_pair[r0:r0 + DK, qc * QC:(qc + 1) * QC],
                            in0=ctx_ps[0:DK, :],
                            in1=recB[0:DK, :],
                            op=AluOpType.mult,
                        )
                        nc.vector.tensor_tensor(
                            out=slab,
                            in0=slab,
                            in1=recB.unsqueeze(1).broadcast_to((P, NS, QC)),
                            op=AluOpType.mult,
                        )
                        nc.sync.dma_start(
                            out=attn_t.rearrange("h m p q -> h p m q")[h][
                                :, :, qc * QC:(qc + 1) * QC
                            ],
                            in_=slab.bitcast(FP32),
                        )
                nc.gpsimd.dma_start(out=ctx_scr[hp], in_=ctx_pair)

        # ---- Output projection + residual + LayerNorm ----
        with tc.tile_pool(name="wts3", bufs=1) as w_pool, \
             tc.tile_pool(name="mm_ps3", bufs=4, space="PSUM") as mm_ps, \
             tc.tile_pool(name="ctxo", bufs=3) as ctxo_pool, \
             tc.tile_pool(name="y", bufs=3) as y_pool, \
             tc.tile_pool(name="xr", bufs=3) as xr_pool, \
             tc.tile_pool(name="lnst", bufs=4) as ln_pool:

            gamB = const.tile([P, D], FP32)
            nc.gpsimd.dma_start(
                out=gamB,
                in_=bass.AP(tensor=gam.tensor, offset=gam.offset, ap=[[0, P], [1, D]]),
            )
            betB = const.tile([P, D], FP32)
            nc.gpsimd.dma_start(
                out=betB,
                in_=bass.AP(tensor=bet.tensor, offset=bet.offset, ap=[[0, P], [1, D]]),
            )
            eps_t = const.tile([P, 1], FP32)
            nc.vector.memset(eps_t, LN_EPS)

            wo_sb = w_pool.tile([P, NF, D], FP32R, tag="w")
            nc.scalar.dma_start(out=wo_sb, in_=r(w_o.rearrange("(k p) f -> p k f", p=P)))

            for qt in range(NSQ):
                ctxo = ctxo_pool.tile([P, NF, P], FP32R)
                nc.scalar.dma_start(
                    out=ctxo,
                    in_=ctx_scr[:, :, qt * P:(qt + 1) * P].rearrange("k p q -> p k q"),
                )
                y = y_pool.tile([P, D], FP32)
                xr = xr_pool.tile([P, D], FP32)
                nc.scalar.dma_start(out=xr, in_=x_q[qt * P:(qt + 1) * P, :])
                for n in range(2):
                    ps = mm_ps.tile([P, QC], FP32)
                    for k in range(NF):
                        nc.tensor.matmul(
                            ps,
                            lhsT=r(ctxo[:, k, :]),
                            rhs=r(wo_sb[:, k, n * QC:(n + 1) * QC]),
                            start=(k == 0),
                            stop=(k == NF - 1),
                        )
                    nc.vector.tensor_tensor(
                        out=y[:, n * QC:(n + 1) * QC],
                        in0=ps,
                        in1=xr[:, n * QC:(n + 1) * QC],
                        op=AluOpType.add,
                    )
                stats = ln_pool.tile([P, 2, nc.vector.BN_STATS_DIM], FP32, tag="stats")
                for g in range(2):
                    nc.vector.bn_stats(out=stats[:, g, :], in_=y[:, g * 512:(g + 1) * 512])
                mv = ln_pool.tile([P, nc.vector.BN_AGGR_DIM], FP32, tag="mv")
                nc.vector.bn_aggr(out=mv, in_=stats)
                sd = ln_pool.tile([P, 1], FP32, tag="sd")
                nc.scalar.activation(
                    out=sd, in_=mv[:, 1:2], func=AF.Sqrt, bias=eps_t, scale=1.0
                )
                rstd = ln_pool.tile([P, 1], FP32, tag="rstd")
                nc.vector.reciprocal(out=rstd, in_=sd)
                t = y_pool.tile([P, D], FP32, tag="t")
                nc.vector.tensor_scalar(
                    out=t,
                    in0=y,
                    scalar1=mv[:, 0:1],
                    scalar2=rstd,
                    op0=AluOpType.subtract,
                    op1=AluOpType.mult,
                )
                nc.vector.tensor_tensor(out=t, in0=t, in1=gamB, op=AluOpType.mult)
                nc.vector.tensor_tensor(out=t, in0=t, in1=betB, op=AluOpType.add)
                nc.sync.dma_start(out=out_q[qt * P:(qt + 1) * P, :], in_=t)

    nc.compile()
    return nc


def get_nc():
    if "nc" not in _CACHE:
        _CACHE["nc"] = _build()
    return _CACHE["nc"]


def make_in_maps(inputs):
    x = np.ascontiguousarray(np.asarray(inputs["x"], dtype=np.float32))
    ws = {
        k: np.ascontiguousarray(np.asarray(inputs[k], dtype=np.float32))
        for k in ("w_q", "w_k", "w_v", "w_o", "ln_gamma", "ln_beta")
    }
    in_maps = []
    for c in range(8):
        b, hf = c // 2, c % 2
        in_maps.append(
            {
                "x_kv": x[b],
                "x_q": np.ascontiguousarray(x[b, hf * NQ:(hf + 1) * NQ]),
                **ws,
            }
        )
    return in_maps


def gather(results):
    out = np.empty((B, S, D), dtype=np.float32)
    attn = np.empty((B, H, S, S), dtype=np.float32)
    for c in range(8):
        b, hf = c // 2, c % 2
        qsl = slice(hf * NQ, (hf + 1) * NQ)
        out[b, qsl] = results[c]["out_q"]
        at = results[c]["attn_t"].reshape(H, S, NQ)  # [h, kk, q]
        attn[b, :, qsl, :] = at.transpose(0, 2, 1)
    return out, attn


def kernel(**inputs):
    nc = get_nc()
    in_maps = make_in_maps(inputs)
    res = run_bass_kernel_spmd(nc, in_maps, core_ids=list(range(8)))
    return gather(res.results)
